# revision 8
# baseline (speedup 1.0000x reference)
"""Trainium2 Bass kernel for nn_LuongAttention.

Reference math (per batch b):
    S   = Dec @ Enc^T          # [T_dec, T_enc]
    Out = S @ Enc              # [T_dec, D]

By associativity:  Out = Dec @ (Enc^T @ Enc) = Dec @ G with G = Enc^T Enc
a [D, D] = [128, 128] Gram matrix.  This removes the [2048, 2048]
intermediate entirely (16x less FLOPs) and makes the kernel
memory-bound: ~1.5 MiB HBM I/O per core at fp16.

Sharding: data-parallel over batch B=8 -> one batch per NeuronCore.

Device-side layout trick: the host feeds Dec pre-transposed (DecT
[D, T]) and receives Out transposed (OutT [D, T]); the host transposes
the result back during the gather (pure layout permutation, no math).
With that:
  - G = sum_i EncTile_i^T @ EncTile_i  (accumulating PE matmuls, natural
    encoder layout - no transposes needed)
  - OutT = G @ DecT computed as matmul(lhsT=G, rhs=DecT chunk) with wide
    moving chunks (G is symmetric so lhsT=G gives G.T@X = G@X)
  - no PE transposes, no identity, minimal PSUM->SBUF copies

Pipelining (from trace analysis):
  - all loads ride the two HWDGE rings; 256 KiB per dma_start (smaller
    transfers cap each ring near 130 GB/s, larger ones near 200+).
  - enc chunks go first on both rings so the Gram matmuls overlap the
    dect stream; dect chunks follow, sized to the 512-wide final
    matmuls that consume them.
  - a ring's FINAL queued transfer completes ~2us late (its tail
    packets trickle once the queue empties); a tiny dummy load queued
    behind the last dect chunk keeps the ring busy so the dect
    completion semaphore fires crisply.
  - junk warmup matmuls keep the PE busy from kernel start through the
    load phase so the HAM clock gate (1.2 -> 2.4 GHz) has a ~3.4us
    sustained-activity window and the real matmuls run warm.
  - each final matmul's PSUM is copied (DVE/ACT alternating) and stored
    immediately, alternating rings, so stores overlap remaining compute.
"""

import os
import sys
from contextlib import ExitStack

import numpy as np

for _p in (
    "/opt/trn_rl_repo",
    "/root/.axon_site",
    "/root/.axon_site/_ro/trn_rl_repo",
    "/root/.axon_site/_ro/pypackages",
):
    if os.path.isdir(_p) and _p not in sys.path:
        sys.path.append(_p)

import concourse.bacc as bacc
import concourse.mybir as mybir
import concourse.tile as tile
from concourse.bass_utils import run_bass_kernel_spmd

B, T, D, P = 8, 2048, 128, 128
NT = T // P  # 16 row tiles of 128

# tunables
ENC_CHUNKS = 2  # enc load chunks (256 KiB each at 2)
DEC_CHUNKS = 2  # dect load chunks
FINAL_N = 512  # moving-operand width of the final matmul (PSUM bank limit)
WARMUP_MMS = 32  # junk matmuls to span kernel start -> first enc sem (~107ns each)
DUMMY_TAIL = True  # tiny load behind the last dect chunk on each ring


def _build_nc():
    nc = bacc.Bacc("TRN2", target_bir_lowering=False, debug=False)
    f32 = mybir.dt.float32
    fp16 = mybir.dt.float16
    in_dt = fp16

    # enc arrives host-pre-shuffled to the SBUF layout [p, n*d] so chunk
    # loads are contiguous per partition.
    enc_h = nc.dram_tensor("enc", [P, NT * D], in_dt, kind="ExternalInput")
    dect_h = nc.dram_tensor("dect", [D, T], in_dt, kind="ExternalInput")
    out_h = nc.dram_tensor("out", [D, T], fp16, kind="ExternalOutput")

    enc_v = enc_h.ap().rearrange("p (n d) -> p n d", d=D)
    dect_v = dect_h.ap()
    out_v = out_h.ap()

    with ExitStack() as ctx:
        tc = ctx.enter_context(tile.TileContext(nc))
        singles = ctx.enter_context(tc.tile_pool(name="singles", bufs=1))
        psum = ctx.enter_context(tc.tile_pool(name="psum", bufs=4, space="PSUM"))
        gpsum = ctx.enter_context(tc.tile_pool(name="gpsum", bufs=2, space="PSUM"))

        enc_sb = singles.tile([P, NT, D], in_dt)
        dect_sb = singles.tile([P, T], in_dt)
        out_sb = singles.tile([P, T], fp16)

        # ---- load issue: enc chunks first (G consumes them), then dect ----
        tpc = NT // ENC_CHUNKS
        for c in range(ENC_CHUNKS):
            eng = nc.sync if c % 2 == 0 else nc.scalar
            eng.dma_start(
                out=enc_sb[:, c * tpc : (c + 1) * tpc, :],
                in_=enc_v[:, c * tpc : (c + 1) * tpc, :],
            )
        # dect rides three concurrent streams: the two HWDGE rings take the
        # head (1024 + 512 cols) behind the enc chunks; the tail 512 cols go
        # over SWDGE, issued immediately (slow ~1us emission + slower data
        # path, but it is consumed last and the long head start keeps it off
        # the critical path) as a third queue pushing the aggregate load
        # rate closer to the 358 GB/s HBM cap.
        wsrc = singles.tile([P, P], in_dt)
        nc.gpsimd.memset(wsrc[:], 0.0)
        nc.sync.dma_start(out=dect_sb[:, 0:1024], in_=dect_v[:, 0:1024])
        nc.scalar.dma_start(out=dect_sb[:, 1024:1536], in_=dect_v[:, 1024:1536])
        nc.gpsimd.dma_start(out=dect_sb[:, 1536:2048], in_=dect_v[:, 1536:2048])
        if DUMMY_TAIL:
            # Keep each queue non-empty past its last dect chunk so the dect
            # completion sems fire without the queue-empty trickle.
            dummy_sb = singles.tile([P, 3, 8], in_dt)
            nc.sync.dma_start(out=dummy_sb[:, 0, :], in_=enc_v[:, 0, :8])
            nc.scalar.dma_start(out=dummy_sb[:, 1, :], in_=enc_v[:, 1, :8])
            nc.gpsimd.dma_start(out=dummy_sb[:, 2, :], in_=enc_v[:, 2, :8])

        # ---- PE warmup: fill the issue->first-enc-sem window (~3us) ----
        if WARMUP_MMS:
            wps = gpsum.tile([P, P], f32, tag="warm")
            for w in range(WARMUP_MMS):
                nc.tensor.matmul(
                    wps[:],
                    lhsT=wsrc[:],
                    rhs=wsrc[:],
                    start=(w == 0),
                    stop=(w == WARMUP_MMS - 1),
                )

        # ---- Gram matrix construction, pipelined with the enc stream ----
        g_sb = singles.tile([P, P], in_dt)
        g_ps = gpsum.tile([P, P], f32, tag="ga")
        for i in range(NT):
            nc.tensor.matmul(
                g_ps[:],
                lhsT=enc_sb[:, i, :],
                rhs=enc_sb[:, i, :],
                start=(i == 0),
                stop=(i == NT - 1),
            )
        nc.vector.tensor_copy(g_sb[:], g_ps[:])

        # ---- OutT = G @ DecT: wide moving chunks, stationary G ----
        n_final = T // FINAL_N
        for c in range(n_final):
            op = psum.tile([P, FINAL_N], f32, tag="op")
            lo = c * FINAL_N
            nc.tensor.matmul(
                op[:],
                lhsT=g_sb[:],
                rhs=dect_sb[:, lo : lo + FINAL_N],
                start=True,
                stop=True,
            )
            if c % 2 == 0:
                nc.vector.tensor_copy(out_sb[:, lo : lo + FINAL_N], op[:])
            else:
                nc.scalar.copy(out_sb[:, lo : lo + FINAL_N], op[:])
            deng = nc.sync if c % 2 == 0 else nc.scalar
            deng.dma_start(
                out=out_v[:, lo : lo + FINAL_N],
                in_=out_sb[:, lo : lo + FINAL_N],
            )

        # Late consumers keep the warmup PSUM and dummy loads alive.  They
        # must NOT ride the DVE queue: the Tile scheduler hoists them ahead
        # of the G cast there, stalling it on the dummy-load semaphores.
        if WARMUP_MMS:
            wsink = singles.tile([P, 1], f32)
            nc.scalar.copy(wsink[:], wps[:, :1])
        if DUMMY_TAIL:
            dsink = singles.tile([P, 3, 8], in_dt)
            nc.gpsimd.tensor_copy(dsink[:], dummy_sb[:])

    nc.compile()
    return nc


_NC = {}


def _get_nc():
    if "nc" not in _NC:
        _NC["nc"] = _build_nc()
    return _NC["nc"]


def _run(enc, dec, **kwargs):
    nc = _get_nc()
    np_dt = np.float16
    in_maps = []
    for b in range(B):
        in_maps.append(
            {
                "enc": np.ascontiguousarray(
                    enc[b].astype(np_dt).reshape(NT, P, D).transpose(1, 0, 2).reshape(P, NT * D)
                ),
                "dect": np.ascontiguousarray(dec[b].T.astype(np_dt)),
            }
        )
    res = run_bass_kernel_spmd(nc, in_maps, core_ids=list(range(B)), **kwargs)
    out = np.stack([res.results[b]["out"].T.astype(np.float32) for b in range(B)], axis=0)
    return np.ascontiguousarray(out), res


def kernel(encoder_hidden_states, decoder_hidden_states):
    enc = np.ascontiguousarray(np.asarray(encoder_hidden_states, dtype=np.float32))
    dec = np.ascontiguousarray(np.asarray(decoder_hidden_states, dtype=np.float32))
    assert enc.shape == (B, T, D) and dec.shape == (B, T, D)
    out, _ = _run(enc, dec)
    return out


# revision 11
# speedup vs baseline: 1.0377x; 1.0377x over previous
"""Trainium2 Bass kernel for nn_LuongAttention.

Reference math (per batch b):
    S   = Dec @ Enc^T          # [T_dec, T_enc]
    Out = S @ Enc              # [T_dec, D]

By associativity:  Out = Dec @ (Enc^T @ Enc) = Dec @ G with G = Enc^T Enc
a [D, D] = [128, 128] Gram matrix.  This removes the [2048, 2048]
intermediate entirely (16x less FLOPs) and makes the kernel
memory-bound: ~1.5 MiB HBM I/O per core at fp16.

Sharding: data-parallel over batch B=8 -> one batch per NeuronCore.

Device-side layout trick: the host feeds Dec pre-transposed (DecT
[D, T]) and receives Out transposed (OutT [D, T]); the host transposes
the result back during the gather (pure layout permutation, no math).
With that:
  - G = sum_i EncTile_i^T @ EncTile_i  (accumulating PE matmuls, natural
    encoder layout - no transposes needed)
  - OutT = G @ DecT computed as matmul(lhsT=G, rhs=DecT chunk) with wide
    moving chunks (G is symmetric so lhsT=G gives G.T@X = G@X)
  - no PE transposes, no identity, minimal PSUM->SBUF copies

Pipelining (from trace analysis):
  - loads ride all THREE DMA queues (the two HWDGE rings + SWDGE): a
    single ring caps near 200 GB/s and two near 300, below the 358 GB/s
    HBM limit, so the third queue shortens the load phase.  enc goes
    first on every queue (the Gram matmuls consume it in queue-arrival
    order); dect follows, split so each 512-wide final matmul's chunk
    completes in consumption order.
  - a queue's FINAL transfer completes ~2us late (its tail packets
    trickle once the queue empties); a tiny dummy load queued behind
    the last dect chunk keeps each queue busy so the dect completion
    semaphores fire crisply.
  - junk warmup matmuls keep the PE busy from kernel start through the
    load phase so the HAM clock gate (1.2 -> 2.4 GHz) has a ~3.4us
    sustained-activity window and the real matmuls run warm; their
    source tile is memset on DVE so the gpsimd queue can start its
    SWDGE transfers immediately.
  - each final matmul's PSUM is copied (DVE/ACT alternating) and stored
    immediately, alternating rings, so stores overlap remaining compute.
"""

import os
import sys
from contextlib import ExitStack

import numpy as np

for _p in (
    "/opt/trn_rl_repo",
    "/root/.axon_site",
    "/root/.axon_site/_ro/trn_rl_repo",
    "/root/.axon_site/_ro/pypackages",
):
    if os.path.isdir(_p) and _p not in sys.path:
        sys.path.append(_p)

import concourse.bacc as bacc
import concourse.mybir as mybir
import concourse.tile as tile
from concourse.bass_utils import run_bass_kernel_spmd

B, T, D, P = 8, 2048, 128, 128
NT = T // P  # 16 row tiles of 128

# tunables
FINAL_N = 512  # moving-operand width of the final matmul (PSUM bank limit)
WARMUP_MMS = 30  # junk matmuls to span kernel start -> first enc sem (~107ns each)
DUMMY_TAIL = True  # tiny load behind the last dect chunk on each queue
# enc row-tile split across the three load queues (sync / scalar / SWDGE);
# sync's ring starts ~0.9us before scalar's, SWDGE's data path is slowest,
# so sync gets the most tiles and the G matmuls consume in this order.
ENC_SPLIT = (7, 4, 5)


def _build_nc():
    nc = bacc.Bacc("TRN2", target_bir_lowering=False, debug=False)
    f32 = mybir.dt.float32
    fp16 = mybir.dt.float16
    in_dt = fp16

    # enc arrives host-pre-shuffled to the SBUF layout [p, n*d] so chunk
    # loads are contiguous per partition.
    enc_h = nc.dram_tensor("enc", [P, NT * D], in_dt, kind="ExternalInput")
    dect_h = nc.dram_tensor("dect", [D, T], in_dt, kind="ExternalInput")
    out_h = nc.dram_tensor("out", [D, T], fp16, kind="ExternalOutput")

    enc_v = enc_h.ap().rearrange("p (n d) -> p n d", d=D)
    dect_v = dect_h.ap()
    out_v = out_h.ap()

    with ExitStack() as ctx:
        tc = ctx.enter_context(tile.TileContext(nc))
        singles = ctx.enter_context(tc.tile_pool(name="singles", bufs=1))
        psum = ctx.enter_context(tc.tile_pool(name="psum", bufs=4, space="PSUM"))
        gpsum = ctx.enter_context(tc.tile_pool(name="gpsum", bufs=2, space="PSUM"))

        enc_sb = singles.tile([P, NT, D], in_dt)
        dect_sb = singles.tile([P, T], in_dt)
        out_sb = singles.tile([P, T], fp16)

        # ---- load issue: enc first on ALL THREE queues (G consumes it),
        # then dect behind it on each queue.  The warmup source memset rides
        # DVE so the gpsimd queue issues its SWDGE transfers immediately.
        wsrc = singles.tile([P, P], in_dt)
        nc.vector.memset(wsrc[:], 0.0)
        e0, e1, e2 = ENC_SPLIT
        nc.sync.dma_start(out=enc_sb[:, 0:e0, :], in_=enc_v[:, 0:e0, :])
        nc.scalar.dma_start(
            out=enc_sb[:, e0 : e0 + e1, :], in_=enc_v[:, e0 : e0 + e1, :]
        )
        nc.gpsimd.dma_start(
            out=enc_sb[:, e0 + e1 :, :], in_=enc_v[:, e0 + e1 :, :]
        )
        # dect: head 1024 cols on sync (earliest ring), then 512 on scalar,
        # tail 512 on SWDGE — each consumed by the finals in that order.
        nc.sync.dma_start(out=dect_sb[:, 0:1024], in_=dect_v[:, 0:1024])
        nc.scalar.dma_start(out=dect_sb[:, 1024:1536], in_=dect_v[:, 1024:1536])
        nc.gpsimd.dma_start(out=dect_sb[:, 1536:2048], in_=dect_v[:, 1536:2048])
        if DUMMY_TAIL:
            # Keep each queue non-empty past its last dect chunk so the dect
            # completion sems fire without the queue-empty trickle.
            dummy_sb = singles.tile([P, 3, 8], in_dt)
            nc.sync.dma_start(out=dummy_sb[:, 0, :], in_=enc_v[:, 0, :8])
            nc.scalar.dma_start(out=dummy_sb[:, 1, :], in_=enc_v[:, 1, :8])
            nc.gpsimd.dma_start(out=dummy_sb[:, 2, :], in_=enc_v[:, 2, :8])

        # ---- PE warmup: fill the issue->first-enc-sem window (~3us) ----
        if WARMUP_MMS:
            wps = gpsum.tile([P, P], f32, tag="warm")
            for w in range(WARMUP_MMS):
                nc.tensor.matmul(
                    wps[:],
                    lhsT=wsrc[:],
                    rhs=wsrc[:],
                    start=(w == 0),
                    stop=(w == WARMUP_MMS - 1),
                )

        # ---- Gram matrix construction, pipelined with the enc stream ----
        g_sb = singles.tile([P, P], in_dt)
        g_ps = gpsum.tile([P, P], f32, tag="ga")
        for i in range(NT):
            nc.tensor.matmul(
                g_ps[:],
                lhsT=enc_sb[:, i, :],
                rhs=enc_sb[:, i, :],
                start=(i == 0),
                stop=(i == NT - 1),
            )
        nc.vector.tensor_copy(g_sb[:], g_ps[:])

        # ---- OutT = G @ DecT: wide moving chunks, stationary G ----
        n_final = T // FINAL_N
        for c in range(n_final):
            op = psum.tile([P, FINAL_N], f32, tag="op")
            lo = c * FINAL_N
            nc.tensor.matmul(
                op[:],
                lhsT=g_sb[:],
                rhs=dect_sb[:, lo : lo + FINAL_N],
                start=True,
                stop=True,
            )
            if c % 2 == 0:
                nc.vector.tensor_copy(out_sb[:, lo : lo + FINAL_N], op[:])
            else:
                nc.scalar.copy(out_sb[:, lo : lo + FINAL_N], op[:])
            deng = nc.sync if c % 2 == 0 else nc.scalar
            deng.dma_start(
                out=out_v[:, lo : lo + FINAL_N],
                in_=out_sb[:, lo : lo + FINAL_N],
            )

        # Late consumers keep the warmup PSUM and dummy loads alive.  They
        # must NOT ride the DVE queue: the Tile scheduler hoists them ahead
        # of the G cast there, stalling it on the dummy-load semaphores.
        if WARMUP_MMS:
            wsink = singles.tile([P, 1], f32)
            nc.scalar.copy(wsink[:], wps[:, :1])
        if DUMMY_TAIL:
            dsink = singles.tile([P, 3, 8], in_dt)
            nc.gpsimd.tensor_copy(dsink[:], dummy_sb[:])

    nc.compile()
    return nc


_NC = {}


def _get_nc():
    if "nc" not in _NC:
        _NC["nc"] = _build_nc()
    return _NC["nc"]


def _run(enc, dec, **kwargs):
    nc = _get_nc()
    np_dt = np.float16
    in_maps = []
    for b in range(B):
        in_maps.append(
            {
                "enc": np.ascontiguousarray(
                    enc[b].astype(np_dt).reshape(NT, P, D).transpose(1, 0, 2).reshape(P, NT * D)
                ),
                "dect": np.ascontiguousarray(dec[b].T.astype(np_dt)),
            }
        )
    res = run_bass_kernel_spmd(nc, in_maps, core_ids=list(range(B)), **kwargs)
    out = np.stack([res.results[b]["out"].T.astype(np.float32) for b in range(B)], axis=0)
    return np.ascontiguousarray(out), res


def kernel(encoder_hidden_states, decoder_hidden_states):
    enc = np.ascontiguousarray(np.asarray(encoder_hidden_states, dtype=np.float32))
    dec = np.ascontiguousarray(np.asarray(decoder_hidden_states, dtype=np.float32))
    assert enc.shape == (B, T, D) and dec.shape == (B, T, D)
    out, _ = _run(enc, dec)
    return out
